# revision 6
# baseline (speedup 1.0000x reference)
"""Multi-head self-attention (B=4, N=2048, D=1024, H=16) on 8 Trainium2 cores.

Sharding: batch (4) x head-group (2 groups of 8 heads) -> 8 cores.
Each core computes, for its batch b and heads [8g, 8g+8):
  qkv = x_b @ w_slice            (projection, fp32r matmuls)
  S^T[n,m] = K Q^T               (scores transposed: keys on partitions)
  E = exp(S^T / 8)               (ScalarE, no max-subtraction needed:
                                  scores ~ N(0,1), max ~ 6, exp safe in fp32)
  out^T[d,m], den[m] = [V|1]^T E (single matmul per n-chunk, fp32r)
  out = transpose(out^T) / den   (PE transpose + DVE normalize)

Device layouts:
  qT, kT  [128, 4, 2048] bf16  : chunk hp holds head 2hp on partitions 0-63
                                 and head 2hp+1 on partitions 64-127
  v_sb    [128, 16, 8, 65] f32r: [n-part, n-chunk, head, head_dim+ones]

All matmuls have free dim 512 (full PE rate for fp32r); the two heads of a
pair run row-packed (K=64 at tile rows 0/64) for the scores matmul.
"""

import numpy as np

import concourse.bacc as bacc
import concourse.bass_utils as bass_utils
import concourse.mybir as mybir
import concourse.tile as tile
from concourse.masks import make_identity

B, N, D = 4, 2048, 1024
H, HD = 16, 64
NCORES = 8
HPC = 8  # heads per core
GW = HPC * HD  # 512, output-column group width per core
P = 128
KO = D // P  # 8 k-chunks of 128
MT = N // 512  # 4 m-tiles of 512
NCH = N // P  # 16 n-chunks of 128
HPAIRS = HPC // 2  # 4 head pairs

F32 = mybir.dt.float32
F32R = mybir.dt.float32r
BF16 = mybir.dt.bfloat16
EXPF = mybir.ActivationFunctionType.Exp

_CACHE: dict = {}


def _emit(nc, tc, x_d, w_d, o_d, n=N):
    MT = n // 512
    NCH = n // P
    with (
        tc.tile_pool(name="constp", bufs=1) as constp,
        tc.tile_pool(name="qkp", bufs=1) as qkp,
        tc.tile_pool(name="vp", bufs=1) as vp,
    ):
        ident = constp.tile([P, P], F32)
        make_identity(nc, ident)

        qT = qkp.tile([P, HPAIRS, n], BF16)
        kT = qkp.tile([P, HPAIRS, n], BF16)
        v_sb = vp.tile([P, NCH, HPC, HD + 1], F32R)
        ones_c = constp.tile([P, 1], F32)
        nc.vector.memset(ones_c, 1.0)
        nc.vector.tensor_copy(v_sb[:, :, :, HD], ones_c.to_broadcast([P, NCH, HPC]))

        # ---- Phase A: QKV projection ----
        with (
            tc.tile_pool(name="wtmp", bufs=2) as wtmp,
            tc.tile_pool(name="wp", bufs=1) as wp,
            tc.tile_pool(name="xp", bufs=3) as xp,
            tc.tile_pool(name="xtp", bufs=2) as xtp,
            tc.tile_pool(name="psA", bufs=4, space="PSUM") as psA,
            tc.tile_pool(name="pstA", bufs=3, space="PSUM") as pstA,
        ):
            # load + round weights to fp32r, one ko-chunk at a time
            w_r = wp.tile([P, KO, 3 * GW], F32R)
            w3 = w_d.rearrange("(ko p) n -> ko p n", p=P)
            for ko in range(KO):
                wt = wtmp.tile([P, 3 * GW], F32, tag="wt")
                nc.sync.dma_start(wt, w3[ko])
                nc.scalar.copy(w_r[:, ko, :], wt)

            for mt in range(MT):
                # transpose x[mt] -> xt [128(k), ko, 512(m)]
                xt = xtp.tile([P, KO, 512], F32R, tag="xt")
                for ms in range(4):
                    xn = xp.tile([P, D], F32, tag="xn")
                    nc.sync.dma_start(xn, x_d[(mt * 4 + ms) * P : (mt * 4 + ms + 1) * P, :])
                    for ko in range(KO):
                        pst = pstA.tile([P, P], F32, tag="pst")
                        nc.tensor.transpose(pst, xn[:, ko * P : (ko + 1) * P], ident)
                        nc.vector.tensor_copy(xt[:, ko, ms * P : (ms + 1) * P], pst)

                # q^T and k^T chunks: psum [dout 128, m 512]
                for qk_idx, dstT in ((0, qT), (1, kT)):
                    for hp in range(HPAIRS):
                        psq = psA.tile([P, 512], F32, tag="psA", name="psq")
                        col0 = qk_idx * GW + hp * P
                        for ko in range(KO):
                            nc.tensor.matmul(
                                psq,
                                lhsT=w_r[:, ko, col0 : col0 + P],
                                rhs=xt[:, ko, :],
                                start=(ko == 0),
                                stop=(ko == KO - 1),
                            )
                        nc.vector.tensor_copy(dstT[:, hp, mt * 512 : (mt + 1) * 512], psq)

                # V: psum [m-sub 128, dv 512]
                for ms in range(4):
                    psv = psA.tile([P, GW], F32, tag="psA", name="psv")
                    for ko in range(KO):
                        nc.tensor.matmul(
                            psv,
                            lhsT=xt[:, ko, ms * P : (ms + 1) * P],
                            rhs=w_r[:, ko, 2 * GW : 3 * GW],
                            start=(ko == 0),
                            stop=(ko == KO - 1),
                        )
                    nc.vector.tensor_copy(
                        v_sb[:, mt * 4 + ms, :, 0:HD],
                        psv.rearrange("p (h d) -> p h d", d=HD),
                    )

        # ---- Phase B: attention ----
        with (
            tc.tile_pool(name="ep", bufs=3) as ep,
            tc.tile_pool(name="otp", bufs=4) as otp,
            tc.tile_pool(name="op", bufs=4) as op,
            tc.tile_pool(name="rp", bufs=4) as rp,
            tc.tile_pool(name="psS", bufs=2, space="PSUM") as psS,
            tc.tile_pool(name="psO", bufs=2, space="PSUM") as psO,
            tc.tile_pool(name="psT", bufs=2, space="PSUM") as psT,
        ):
            for hp in range(HPAIRS):
                for mt in range(MT):
                    mres = slice(mt * 512, (mt + 1) * 512)
                    po0 = psO.tile([HD + 1, 512], F32, tag="po", name="po0")
                    po1 = psO.tile([HD + 1, 512], F32, tag="po", name="po1")
                    for nch in range(NCH):
                        nres = slice(nch * P, (nch + 1) * P)
                        pss = psS.tile([P, 1024], F32, tag="pss")
                        # scores^T for the head pair, row-packed K=64
                        nc.tensor.matmul(
                            pss[:, 0:512],
                            lhsT=kT[0:64, hp, nres],
                            rhs=qT[0:64, hp, mres],
                            start=True,
                            stop=True,
                        )
                        nc.tensor.matmul(
                            pss[:, 512:1024],
                            lhsT=kT[64:128, hp, nres],
                            rhs=qT[64:128, hp, mres],
                            start=True,
                            stop=True,
                        )
                        e = ep.tile([P, 1024], F32R, tag="e")
                        nc.scalar.activation(e, pss, EXPF, scale=0.125)
                        # out^T accumulation: [V|1]^T @ E
                        nc.tensor.matmul(
                            po0,
                            lhsT=v_sb[:, nch, 2 * hp, :],
                            rhs=e[:, 0:512],
                            start=(nch == 0),
                            stop=(nch == NCH - 1),
                        )
                        nc.tensor.matmul(
                            po1,
                            lhsT=v_sb[:, nch, 2 * hp + 1, :],
                            rhs=e[:, 512:1024],
                            start=(nch == 0),
                            stop=(nch == NCH - 1),
                        )
                    ot0 = otp.tile([HD + 1, 512], F32, tag="ot", name="ot0")
                    ot1 = otp.tile([HD + 1, 512], F32, tag="ot", name="ot1")
                    nc.vector.tensor_copy(ot0, po0)
                    nc.vector.tensor_copy(ot1, po1)
                    for ms in range(4):
                        o2 = op.tile([P, P], F32, tag="o2")
                        for h01, ot in ((0, ot0), (1, ot1)):
                            pt = psT.tile([P, P], F32, tag="pt")
                            nc.tensor.transpose(
                                pt[:, 0 : HD + 1],
                                ot[:, ms * P : (ms + 1) * P],
                                ident[0 : HD + 1, 0 : HD + 1],
                            )
                            r = rp.tile([P, 1], F32, tag="r")
                            nc.vector.reciprocal(r, pt[:, HD : HD + 1])
                            nc.vector.tensor_mul(
                                out=o2[:, h01 * HD : (h01 + 1) * HD],
                                in0=pt[:, 0:HD],
                                in1=r.to_broadcast([P, HD]),
                            )
                        nc.sync.dma_start(
                            o_d[(mt * 4 + ms) * P : (mt * 4 + ms + 1) * P, hp * P : (hp + 1) * P],
                            o2,
                        )


def build(n=N, num_devices=NCORES, reps=1):
    key = (n, num_devices, reps)
    if key in _CACHE:
        return _CACHE[key]
    nc = bacc.Bacc("TRN2", target_bir_lowering=False, debug=False, num_devices=num_devices)
    x_d = nc.dram_tensor("x_s", [n, D], F32, kind="ExternalInput").ap()
    w_d = nc.dram_tensor("w_s", [D, 3 * GW], F32, kind="ExternalInput").ap()
    o_d = nc.dram_tensor("o_s", [n, GW], F32, kind="ExternalOutput").ap()
    with tile.TileContext(nc) as tc:
        for _ in range(reps):
            _emit(nc, tc, x_d, w_d, o_d, n=n)
    nc.compile()
    _CACHE[key] = nc
    return nc


def make_in_maps(x, w_qkv):
    x = np.asarray(x, dtype=np.float32)
    w_qkv = np.asarray(w_qkv, dtype=np.float32)
    in_maps = []
    for c in range(NCORES):
        b, g = divmod(c, 2)
        xs = np.ascontiguousarray(x[b])
        ws = np.ascontiguousarray(
            np.concatenate(
                [
                    w_qkv[:, g * GW : (g + 1) * GW],
                    w_qkv[:, D + g * GW : D + (g + 1) * GW],
                    w_qkv[:, 2 * D + g * GW : 2 * D + (g + 1) * GW],
                ],
                axis=1,
            )
        )
        in_maps.append({"x_s": xs, "w_s": ws})
    return in_maps


def assemble(results):
    out = np.empty((B, N, D), np.float32)
    for c in range(NCORES):
        b, g = divmod(c, 2)
        out[b][:, g * GW : (g + 1) * GW] = results[c]["o_s"]
    return out


def kernel(x, w_qkv, **run_kwargs):
    nc = build()
    in_maps = make_in_maps(x, w_qkv)
    res = bass_utils.run_bass_kernel_spmd(
        nc, in_maps, core_ids=list(range(NCORES)), **run_kwargs
    )
    out = assemble(res.results)
    if run_kwargs:
        kernel.last_result = res
    return out
